# revision 9
# baseline (speedup 1.0000x reference)
"""EdgeConv (gnn_message_passing) Trainium2 Bass kernel, v2.

Computation (reference):
    neigh = x[ind]                                   # [n, k, d] gather
    feat  = [neigh - center, center]                 # [n, k, 2d]
    h     = relu(feat @ W1 + b1) @ W2 + b2           # [n, k, H]
    out   = max over k                               # [n, H]

Algebraic restructuring:
    feat @ W1 + b1 = P[j] + u[n],   P = x @ W1[:d],  u = x @ (W1[d:]-W1[:d]) + b1
    relu(P + u)    = u + max(P, -u)
    h              = max(P, -u) @ W2 + (u @ W2 + b2) = mm2(g) + v[n]
    out[n]         = max_k mm2(g)[n,k] + v[n]
P (the per-point first-layer neighbor term), -u and v are precomputed on the
host; the device gathers P rows, takes an elementwise max against the
broadcast center term, runs one matmul, and max-pools.

The dma_gather index operand is int16, so a 100k-row table is not directly
addressable.  The host therefore compacts the table per megablock:
comp[m] = P[unique(ind rows of megablock m)] (~13.4k rows), and indices are
renumbered to table-local ranks.  One transpose-mode dma_gather per megablock
then fetches all 14336 edge columns feature-major in canonical (point, k)
order.

Per-core per-megablock dataflow (data-parallel over points, 8 cores):
    dma_gather(transpose) -> slabT [128=H, E]
    DVE tensor_max(slabT, -u broadcast over k)       # stride-0 AP
    PE matmul (W2 stationary) -> PSUM
    DVE tensor_reduce(max over k) -> +v -> DMA out
Output is produced transposed ([H, points]); the host transposes back.
"""

import os
import sys

for _p in ("/opt/trn_rl_repo",):
    if _p not in sys.path and os.path.isdir(_p):
        sys.path.insert(0, _p)

import numpy as np
import ml_dtypes

BF16 = ml_dtypes.bfloat16

# problem constants (hardcoded per harness contract)
N, D, K, H = 100000, 64, 16, 128
NCORES = 8
NP = 12500             # points per core
MEGA = 896             # points per megablock


class Cfg:
    def __init__(self, n=N, np_=NP, mega=MEGA, ct=None):
        self.n = n
        self.np = np_                   # points handled by this core (unpadded)
        self.mega = mega                # points per megablock
        self.e = mega * K               # edges per megablock
        assert self.e % 512 == 0
        self.nmega = -(-np_ // mega)    # ceil
        self.npp = self.nmega * mega    # padded points per core
        self.ct = ct                    # compact-table rows per megablock
        if ct is not None:
            assert ct < 32768


def build_program(cfg: Cfg, dump=False):
    """Build the per-core Bass program (same program for every core)."""
    import concourse.bacc as bacc
    import concourse.bass as bass
    import concourse.tile as tile
    from concourse import mybir

    f32 = mybir.dt.float32
    bf16 = mybir.dt.bfloat16
    i16 = mybir.dt.int16
    E = cfg.e
    CT = cfg.ct

    nc = bacc.Bacc("TRN2", target_bir_lowering=False, debug=False,
                   num_swdge_queues=4)

    comp = nc.dram_tensor("comp", (cfg.nmega, CT, H), bf16,
                          kind="ExternalInput")
    gi = nc.dram_tensor("gi", (cfg.nmega, 128, E // 16), i16,
                        kind="ExternalInput")
    nu = nc.dram_tensor("nu", (cfg.nmega, H, cfg.mega), bf16,
                        kind="ExternalInput")
    vt = nc.dram_tensor("vt", (cfg.nmega, H, cfg.mega), f32,
                        kind="ExternalInput")
    w2 = nc.dram_tensor("w2", (H, H), bf16, kind="ExternalInput")
    out2 = nc.dram_tensor("out2", (H, cfg.npp), f32, kind="ExternalOutput")
    if dump:
        d_slabT = nc.dram_tensor("d_slabT", (128, E), bf16,
                                 kind="ExternalOutput")
        d_g1t = nc.dram_tensor("d_g1t", (128, 512), bf16,
                               kind="ExternalOutput")

    NI = 512   # idxs per transpose-mode gather call
    MMC = 512               # matmul columns per PE call (one PSUM bank)
    with tile.TileContext(nc) as tc:
        with (
            tc.tile_pool(name="const", bufs=1) as constp,
            tc.tile_pool(name="gi", bufs=2) as gip,
            tc.tile_pool(name="nuv", bufs=2) as nuvp,
            tc.tile_pool(name="slabT", bufs=2) as slabTp,
            tc.tile_pool(name="g1t", bufs=2) as g1tp,
            tc.tile_pool(name="h2", bufs=4) as h2p,
            tc.tile_pool(name="mx", bufs=2) as mxp,
            tc.tile_pool(name="outs", bufs=2) as outp,
            tc.tile_pool(name="ps", bufs=4, space="PSUM") as psp,
        ):
            w2s = constp.tile([H, H], bf16)
            nc.sync.dma_start(w2s[:], w2[:, :])

            for m in range(cfg.nmega):
                i2 = gip.tile([128, E // 16], i16)
                nc.sync.dma_start(i2[:], gi[m])
                nut = nuvp.tile([H, cfg.mega], bf16, tag="nut")
                nc.sync.dma_start(nut[:], nu[m])
                vtt = nuvp.tile([H, cfg.mega], f32, tag="vtt")
                nc.sync.dma_start(vtt[:], vt[m])

                slabT = slabTp.tile([128, E], bf16)
                for j in range(E // NI):
                    nc.gpsimd.dma_gather(
                        slabT[:, j * NI:(j + 1) * NI].rearrange(
                            "p (a b) -> p a b", a=1),
                        comp[m, :, :],
                        i2[:, j * (NI // 16):(j + 1) * (NI // 16)],
                        num_idxs=NI,
                        num_idxs_reg=NI,
                        elem_size=H,
                        transpose=True,
                        queue_num=0,
                    )
                if dump and m == 0:
                    nc.sync.dma_start(d_slabT[:, :], slabT[:])

                # g = max(P_gathered, -u) in one fused DVE op per megablock
                g1T = g1tp.tile([128, E], bf16)
                nu3 = bass.AP(nut.tensor, nut[:].offset,
                              list(nut[:].ap) + [(0, K)])
                nc.vector.tensor_max(
                    g1T[:].rearrange("p (a b) -> p a b", b=K),
                    slabT[:].rearrange("p (a b) -> p a b", b=K),
                    nu3)
                if dump and m == 0:
                    nc.sync.dma_start(d_g1t[:, :], g1T[:, 0:512])

                mx = mxp.tile([H, cfg.mega], f32)
                for g in range(E // MMC):
                    p2 = psp.tile([H, MMC], f32)
                    nc.tensor.matmul(
                        p2[:], lhsT=w2s[:],
                        rhs=g1T[:, MMC * g:MMC * (g + 1)],
                        start=True, stop=True,
                    )
                    h2 = h2p.tile([H, MMC], bf16)
                    nc.scalar.activation(
                        h2[:], p2[:], mybir.ActivationFunctionType.Copy,
                        scale=1.0,
                    )
                    nc.vector.tensor_reduce(
                        out=mx[:, (MMC // K) * g:(MMC // K) * (g + 1)],
                        in_=h2[:].rearrange("p (a b) -> p a b", b=K),
                        axis=mybir.AxisListType.X,
                        op=mybir.AluOpType.max,
                    )

                outt = outp.tile([H, cfg.mega], f32)
                nc.vector.tensor_tensor(
                    out=outt[:], in0=mx[:], in1=vtt[:],
                    op=mybir.AluOpType.add,
                )
                nc.sync.dma_start(
                    out2[:, m * cfg.mega:(m + 1) * cfg.mega], outt[:]
                )

    nc.compile()
    return nc


def host_prep(cfg: Cfg, x, ind, W1, b1, W2, b2):
    """Shared (core-independent) input prep. Returns (ptab, negu, v, w2b)."""
    x = np.asarray(x, np.float32)
    W1 = np.asarray(W1, np.float32)
    W2 = np.asarray(W2, np.float32)
    b1 = np.asarray(b1, np.float32)
    b2 = np.asarray(b2, np.float32)
    P = (x @ W1[:D]).astype(BF16)                        # [n, H] gather table
    u = x @ (W1[D:] - W1[:D]) + b1                       # [n, H] fp32
    negu = (-u).astype(BF16)                             # [n, H]
    v = (u @ W2 + b2).astype(np.float32)                 # [n, H]
    w2b = np.ascontiguousarray(W2.astype(BF16))
    return np.ascontiguousarray(P), negu, v, w2b


def _wrap_idx(idxs):
    """[..., n] int -> dma_gather idx tile [..., 128, n/16] int16
    (idx i at [i%16, i//16], replicated across the 8 partition groups)."""
    n = idxs.shape[-1]
    w = idxs.reshape(*idxs.shape[:-1], n // 16, 16)
    w = np.moveaxis(w, -1, -2)                  # [..., 16, n/16]
    return np.ascontiguousarray(
        np.broadcast_to(
            w[..., None, :, :],
            (*idxs.shape[:-1], 8, 16, n // 16),
        ).reshape(*idxs.shape[:-1], 128, n // 16).astype(np.int16))


def core_tables(cfg: Cfg, ptab, ind32, lo, hi):
    """Per-megablock compact tables + local idx for point range [lo, hi)."""
    E = cfg.e
    indc = np.zeros((cfg.npp, K), np.int32)
    indc[:hi - lo] = ind32[lo:hi]
    flat = indc.reshape(cfg.nmega, E)
    uniqs, ranks = [], np.empty((cfg.nmega, E), np.int64)
    for m in range(cfg.nmega):
        u_m, inv = np.unique(flat[m], return_inverse=True)
        uniqs.append(u_m)
        ranks[m] = inv
    return uniqs, ranks


def core_inputs(cfg: Cfg, ptab, negu, v, w2b, uniqs, ranks, lo, hi):
    comp = np.zeros((cfg.nmega, cfg.ct, H), BF16)
    for m in range(cfg.nmega):
        comp[m, :len(uniqs[m])] = ptab[uniqs[m]]
    gi = _wrap_idx(ranks)                                # [m, 128, E/16]

    nuc = np.zeros((cfg.npp, H), BF16)
    nuc[:hi - lo] = negu[lo:hi]
    nu = np.ascontiguousarray(
        nuc.reshape(cfg.nmega, cfg.mega, H).transpose(0, 2, 1))
    vc = np.zeros((cfg.npp, H), np.float32)
    vc[:hi - lo] = v[lo:hi]
    vtc = np.ascontiguousarray(
        vc.reshape(cfg.nmega, cfg.mega, H).transpose(0, 2, 1))
    return {"comp": comp, "gi": gi, "nu": nu, "vt": vtc, "w2": w2b}


_NC_CACHE = {}


def kernel(x, ind, W1, b1, W2, b2):
    from concourse import bass_utils

    ind32 = np.asarray(ind).astype(np.int32)
    cfg = Cfg()
    ptab, negu, v, w2b = host_prep(cfg, x, ind32, W1, b1, W2, b2)

    per_core = []
    maxct = 0
    for c in range(NCORES):
        lo = c * NP
        hi = min(lo + NP, N)
        uniqs, ranks = core_tables(cfg, ptab, ind32, lo, hi)
        per_core.append((uniqs, ranks, lo, hi))
        maxct = max(maxct, max(len(u) for u in uniqs))
    cfg.ct = maxct

    key = (cfg.n, cfg.np, cfg.mega, cfg.ct)
    if key not in _NC_CACHE:
        _NC_CACHE[key] = build_program(cfg)
    nc = _NC_CACHE[key]

    in_maps = []
    for uniqs, ranks, lo, hi in per_core:
        in_maps.append(
            core_inputs(cfg, ptab, negu, v, w2b, uniqs, ranks, lo, hi))

    res = bass_utils.run_bass_kernel_spmd(nc, in_maps, core_ids=list(range(NCORES)))
    out = np.empty((N, H), np.float32)
    for c in range(NCORES):
        lo = c * NP
        hi = min(lo + NP, N)
        out[lo:hi] = res.results[c]["out2"].T[:hi - lo]
    return out


# revision 11
# speedup vs baseline: 1.6088x; 1.6088x over previous
"""EdgeConv (gnn_message_passing) Trainium2 Bass kernel, v2.

Computation (reference):
    neigh = x[ind]                                   # [n, k, d] gather
    feat  = [neigh - center, center]                 # [n, k, 2d]
    h     = relu(feat @ W1 + b1) @ W2 + b2           # [n, k, H]
    out   = max over k                               # [n, H]

Algebraic restructuring:
    feat @ W1 + b1 = P[j] + u[n],   P = x @ W1[:d],  u = x @ (W1[d:]-W1[:d]) + b1
    relu(P + u)    = u + max(P, -u)
    h              = max(P, -u) @ W2 + (u @ W2 + b2) = mm2(g) + v[n]
    out[n]         = max_k mm2(g)[n,k] + v[n]
P (the per-point first-layer neighbor term), -u and v are precomputed on the
host; the device gathers P rows, takes an elementwise max against the
broadcast center term, runs one matmul, and max-pools.

The dma_gather index operand is int16, so a 100k-row table is not directly
addressable.  The host therefore compacts the table per megablock:
comp[m] = P[unique(ind rows of megablock m)] (~13.4k rows), and indices are
renumbered to table-local ranks.  One transpose-mode dma_gather per megablock
then fetches all 14336 edge columns feature-major in canonical (point, k)
order.

Per-core per-megablock dataflow (data-parallel over points, 8 cores):
    dma_gather(transpose) -> slabT [128=H, E]
    DVE tensor_max(slabT, -u broadcast over k)       # stride-0 AP
    PE matmul (W2 stationary) -> PSUM
    DVE tensor_reduce(max over k) -> +v -> DMA out
Output is produced transposed ([H, points]); the host transposes back.
"""

import os
import sys

for _p in ("/opt/trn_rl_repo",):
    if _p not in sys.path and os.path.isdir(_p):
        sys.path.insert(0, _p)

import numpy as np
import ml_dtypes

BF16 = ml_dtypes.bfloat16

# problem constants (hardcoded per harness contract)
N, D, K, H = 100000, 64, 16, 128
NCORES = 8
NP = 12500             # points per core
MEGA = 896             # points per megablock


class Cfg:
    def __init__(self, n=N, np_=NP, mega=MEGA, ct=None):
        self.n = n
        self.np = np_                   # points handled by this core (unpadded)
        self.mega = mega                # points per megablock
        self.e = mega * K               # edges per megablock
        assert self.e % 512 == 0
        self.nmega = -(-np_ // mega)    # ceil
        self.npp = self.nmega * mega    # padded points per core
        self.ct = ct                    # compact-table rows per megablock
        if ct is not None:
            assert ct < 32768


def build_program(cfg: Cfg, dump=False):
    """Build the per-core Bass program (same program for every core)."""
    import concourse.bacc as bacc
    import concourse.bass as bass
    import concourse.tile as tile
    from concourse import mybir

    f32 = mybir.dt.float32
    bf16 = mybir.dt.bfloat16
    i16 = mybir.dt.int16
    E = cfg.e
    CT = cfg.ct

    nc = bacc.Bacc("TRN2", target_bir_lowering=False, debug=False,
                   num_swdge_queues=4)

    comp = nc.dram_tensor("comp", (cfg.nmega, CT, H), bf16,
                          kind="ExternalInput")
    gi = nc.dram_tensor("gi", (cfg.nmega, 128, E // 16), i16,
                        kind="ExternalInput")
    nu = nc.dram_tensor("nu", (cfg.nmega, H, cfg.mega), bf16,
                        kind="ExternalInput")
    vt = nc.dram_tensor("vt", (cfg.nmega, H, cfg.mega), f32,
                        kind="ExternalInput")
    w2 = nc.dram_tensor("w2", (H, H), bf16, kind="ExternalInput")
    out2 = nc.dram_tensor("out2", (H, cfg.npp), f32, kind="ExternalOutput")
    if dump:
        d_slabT = nc.dram_tensor("d_slabT", (128, E), bf16,
                                 kind="ExternalOutput")
        d_g1t = nc.dram_tensor("d_g1t", (128, 512), bf16,
                               kind="ExternalOutput")

    NI = 1024  # idxs per gather call
    MMC = 512               # matmul columns per PE call (one PSUM bank)
    with tile.TileContext(nc) as tc:
        with (
            tc.tile_pool(name="const", bufs=1) as constp,
            tc.tile_pool(name="gi", bufs=2) as gip,
            tc.tile_pool(name="nuv", bufs=2) as nuvp,
            tc.tile_pool(name="slab", bufs=2) as slabp,
            tc.tile_pool(name="slabT", bufs=2) as slabTp,
            tc.tile_pool(name="g1t", bufs=2) as g1tp,
            tc.tile_pool(name="h2", bufs=4) as h2p,
            tc.tile_pool(name="mx", bufs=2) as mxp,
            tc.tile_pool(name="outs", bufs=2) as outp,
            tc.tile_pool(name="ps", bufs=4, space="PSUM") as psp,
        ):
            w2s = constp.tile([H, H], bf16)
            nc.sync.dma_start(w2s[:], w2[:, :])

            for m in range(cfg.nmega):
                i2 = gip.tile([128, E // 16], i16)
                nc.sync.dma_start(i2[:], gi[m])
                nut = nuvp.tile([H, cfg.mega], bf16, tag="nut")
                nc.sync.dma_start(nut[:], nu[m])
                vtt = nuvp.tile([H, cfg.mega], f32, tag="vtt")
                nc.sync.dma_start(vtt[:], vt[m])

                slab = slabp.tile([128, E // 128, H], bf16)
                for j in range(E // NI):
                    nc.gpsimd.dma_gather(
                        slab[:, j * (NI // 128):(j + 1) * (NI // 128), :],
                        comp[m, :, :],
                        i2[:, j * (NI // 16):(j + 1) * (NI // 16)],
                        num_idxs=NI,
                        num_idxs_reg=NI,
                        elem_size=H,
                        queue_num=j % 4,
                    )
                slabT = slabTp.tile([128, E], bf16)
                nc.sync.dma_start_transpose(
                    slabT[:].rearrange("p (a b) -> p a b", b=H),
                    slab[:].rearrange("p a b -> p (a b)"),
                )
                if dump and m == 0:
                    nc.sync.dma_start(d_slabT[:, :], slabT[:])

                # g = max(P_gathered, -u) in one fused DVE op per megablock
                g1T = g1tp.tile([128, E], bf16)
                nu3 = bass.AP(nut.tensor, nut[:].offset,
                              list(nut[:].ap) + [(0, K)])
                nc.vector.tensor_max(
                    g1T[:].rearrange("p (a b) -> p a b", b=K),
                    slabT[:].rearrange("p (a b) -> p a b", b=K),
                    nu3)
                if dump and m == 0:
                    nc.sync.dma_start(d_g1t[:, :], g1T[:, 0:512])

                mx = mxp.tile([H, cfg.mega], f32)
                for g in range(E // MMC):
                    p2 = psp.tile([H, MMC], f32)
                    nc.tensor.matmul(
                        p2[:], lhsT=w2s[:],
                        rhs=g1T[:, MMC * g:MMC * (g + 1)],
                        start=True, stop=True,
                    )
                    h2 = h2p.tile([H, MMC], bf16)
                    nc.scalar.activation(
                        h2[:], p2[:], mybir.ActivationFunctionType.Copy,
                        scale=1.0,
                    )
                    nc.vector.tensor_reduce(
                        out=mx[:, (MMC // K) * g:(MMC // K) * (g + 1)],
                        in_=h2[:].rearrange("p (a b) -> p a b", b=K),
                        axis=mybir.AxisListType.X,
                        op=mybir.AluOpType.max,
                    )

                outt = outp.tile([H, cfg.mega], f32)
                nc.vector.tensor_tensor(
                    out=outt[:], in0=mx[:], in1=vtt[:],
                    op=mybir.AluOpType.add,
                )
                nc.sync.dma_start(
                    out2[:, m * cfg.mega:(m + 1) * cfg.mega], outt[:]
                )

    nc.compile()
    return nc


def host_prep(cfg: Cfg, x, ind, W1, b1, W2, b2):
    """Shared (core-independent) input prep. Returns (ptab, negu, v, w2b)."""
    x = np.asarray(x, np.float32)
    W1 = np.asarray(W1, np.float32)
    W2 = np.asarray(W2, np.float32)
    b1 = np.asarray(b1, np.float32)
    b2 = np.asarray(b2, np.float32)
    P = (x @ W1[:D]).astype(BF16)                        # [n, H] gather table
    u = x @ (W1[D:] - W1[:D]) + b1                       # [n, H] fp32
    negu = (-u).astype(BF16)                             # [n, H]
    v = (u @ W2 + b2).astype(np.float32)                 # [n, H]
    w2b = np.ascontiguousarray(W2.astype(BF16))
    return np.ascontiguousarray(P), negu, v, w2b


def _wrap_idx(idxs):
    """[..., n] int -> dma_gather idx tile [..., 128, n/16] int16
    (idx i at [i%16, i//16], replicated across the 8 partition groups)."""
    n = idxs.shape[-1]
    w = idxs.reshape(*idxs.shape[:-1], n // 16, 16)
    w = np.moveaxis(w, -1, -2)                  # [..., 16, n/16]
    return np.ascontiguousarray(
        np.broadcast_to(
            w[..., None, :, :],
            (*idxs.shape[:-1], 8, 16, n // 16),
        ).reshape(*idxs.shape[:-1], 128, n // 16).astype(np.int16))


def core_tables(cfg: Cfg, ptab, ind32, lo, hi):
    """Per-megablock compact tables + local idx for point range [lo, hi)."""
    E = cfg.e
    indc = np.zeros((cfg.npp, K), np.int32)
    indc[:hi - lo] = ind32[lo:hi]
    flat = indc.reshape(cfg.nmega, E)
    uniqs, ranks = [], np.empty((cfg.nmega, E), np.int64)
    for m in range(cfg.nmega):
        u_m, inv = np.unique(flat[m], return_inverse=True)
        uniqs.append(u_m)
        ranks[m] = inv
    return uniqs, ranks


def core_inputs(cfg: Cfg, ptab, negu, v, w2b, uniqs, ranks, lo, hi):
    comp = np.zeros((cfg.nmega, cfg.ct, H), BF16)
    for m in range(cfg.nmega):
        comp[m, :len(uniqs[m])] = ptab[uniqs[m]]
    gi = _wrap_idx(ranks)                                # [m, 128, E/16]

    nuc = np.zeros((cfg.npp, H), BF16)
    nuc[:hi - lo] = negu[lo:hi]
    nu = np.ascontiguousarray(
        nuc.reshape(cfg.nmega, cfg.mega, H).transpose(0, 2, 1))
    vc = np.zeros((cfg.npp, H), np.float32)
    vc[:hi - lo] = v[lo:hi]
    vtc = np.ascontiguousarray(
        vc.reshape(cfg.nmega, cfg.mega, H).transpose(0, 2, 1))
    return {"comp": comp, "gi": gi, "nu": nu, "vt": vtc, "w2": w2b}


_NC_CACHE = {}


def kernel(x, ind, W1, b1, W2, b2):
    from concourse import bass_utils

    ind32 = np.asarray(ind).astype(np.int32)
    cfg = Cfg()
    ptab, negu, v, w2b = host_prep(cfg, x, ind32, W1, b1, W2, b2)

    per_core = []
    maxct = 0
    for c in range(NCORES):
        lo = c * NP
        hi = min(lo + NP, N)
        uniqs, ranks = core_tables(cfg, ptab, ind32, lo, hi)
        per_core.append((uniqs, ranks, lo, hi))
        maxct = max(maxct, max(len(u) for u in uniqs))
    cfg.ct = maxct

    key = (cfg.n, cfg.np, cfg.mega, cfg.ct)
    if key not in _NC_CACHE:
        _NC_CACHE[key] = build_program(cfg)
    nc = _NC_CACHE[key]

    in_maps = []
    for uniqs, ranks, lo, hi in per_core:
        in_maps.append(
            core_inputs(cfg, ptab, negu, v, w2b, uniqs, ranks, lo, hi))

    res = bass_utils.run_bass_kernel_spmd(nc, in_maps, core_ids=list(range(NCORES)))
    out = np.empty((N, H), np.float32)
    for c in range(NCORES):
        lo = c * NP
        hi = min(lo + NP, N)
        out[lo:hi] = res.results[c]["out2"].T[:hi - lo]
    return out


# revision 13
# speedup vs baseline: 2.1244x; 1.3205x over previous
"""EdgeConv (gnn_message_passing) Trainium2 Bass kernel, v2.

Computation (reference):
    neigh = x[ind]                                   # [n, k, d] gather
    feat  = [neigh - center, center]                 # [n, k, 2d]
    h     = relu(feat @ W1 + b1) @ W2 + b2           # [n, k, H]
    out   = max over k                               # [n, H]

Algebraic restructuring:
    feat @ W1 + b1 = P[j] + u[n],   P = x @ W1[:d],  u = x @ (W1[d:]-W1[:d]) + b1
    relu(P + u)    = u + max(P, -u)
    h              = max(P, -u) @ W2 + (u @ W2 + b2) = mm2(g) + v[n]
    out[n]         = max_k mm2(g)[n,k] + v[n]
P (the per-point first-layer neighbor term), -u and v are precomputed on the
host; the device gathers P rows, takes an elementwise max against the
broadcast center term, runs one matmul, and max-pools.

The dma_gather index operand is int16, so a 100k-row table is not directly
addressable.  The host therefore compacts the table per megablock:
comp[m] = P[unique(ind rows of megablock m)] (~13.4k rows), and indices are
renumbered to table-local ranks.  One transpose-mode dma_gather per megablock
then fetches all 14336 edge columns feature-major in canonical (point, k)
order.

Per-core per-megablock dataflow (data-parallel over points, 8 cores):
    dma_gather(transpose) -> slabT [128=H, E]
    DVE tensor_max(slabT, -u broadcast over k)       # stride-0 AP
    PE matmul (W2 stationary) -> PSUM
    DVE tensor_reduce(max over k) -> +v -> DMA out
Output is produced transposed ([H, points]); the host transposes back.
"""

import os
import sys

for _p in ("/opt/trn_rl_repo",):
    if _p not in sys.path and os.path.isdir(_p):
        sys.path.insert(0, _p)

import numpy as np
import ml_dtypes

BF16 = ml_dtypes.bfloat16

# problem constants (hardcoded per harness contract)
N, D, K, H = 100000, 64, 16, 128
NCORES = 8
NP = 12500             # points per core
MEGA = 896             # points per megablock


class Cfg:
    def __init__(self, n=N, np_=NP, mega=MEGA, ct=None):
        self.n = n
        self.np = np_                   # points handled by this core (unpadded)
        self.mega = mega                # points per megablock
        self.e = mega * K               # edges per megablock
        assert self.e % 512 == 0
        self.nmega = -(-np_ // mega)    # ceil
        self.npp = self.nmega * mega    # padded points per core
        self.ct = ct                    # compact-table rows per megablock
        if ct is not None:
            assert ct < 32768


def build_program(cfg: Cfg, dump=False):
    """Build the per-core Bass program (same program for every core)."""
    import concourse.bacc as bacc
    import concourse.bass as bass
    import concourse.tile as tile
    from concourse import mybir

    f32 = mybir.dt.float32
    bf16 = mybir.dt.bfloat16
    i16 = mybir.dt.int16
    E = cfg.e
    CT = cfg.ct

    nc = bacc.Bacc("TRN2", target_bir_lowering=False, debug=False,
                   num_swdge_queues=4)

    comp = nc.dram_tensor("comp", (cfg.nmega, CT, H), bf16,
                          kind="ExternalInput")
    gi = nc.dram_tensor("gi", (cfg.nmega, 128, E // 16), i16,
                        kind="ExternalInput")
    nu = nc.dram_tensor("nu", (cfg.nmega, H, cfg.mega), bf16,
                        kind="ExternalInput")
    vt = nc.dram_tensor("vt", (cfg.nmega, H, cfg.mega), bf16,
                        kind="ExternalInput")
    w2 = nc.dram_tensor("w2", (H, H), bf16, kind="ExternalInput")
    out2 = nc.dram_tensor("out2", (H, cfg.npp), bf16, kind="ExternalOutput")
    if dump:
        d_slabT = nc.dram_tensor("d_slabT", (128, E), bf16,
                                 kind="ExternalOutput")
        d_g1t = nc.dram_tensor("d_g1t", (128, 512), bf16,
                               kind="ExternalOutput")

    NI = 1024  # idxs per gather call
    MMC = 512               # matmul columns per PE call (one PSUM bank)
    with tile.TileContext(nc) as tc:
        with (
            tc.tile_pool(name="const", bufs=1) as constp,
            tc.tile_pool(name="gi", bufs=2) as gip,
            tc.tile_pool(name="nuv", bufs=2) as nuvp,
            tc.tile_pool(name="slab", bufs=2) as slabp,
            tc.tile_pool(name="slabT", bufs=2) as slabTp,
            tc.tile_pool(name="g1t", bufs=4) as g1tp,
            tc.tile_pool(name="mx", bufs=2) as mxp,
            tc.tile_pool(name="outs", bufs=2) as outp,
            tc.tile_pool(name="ps", bufs=4, space="PSUM") as psp,
        ):
            w2s = constp.tile([H, H], bf16)
            nc.sync.dma_start(w2s[:], w2[:, :])

            for m in range(cfg.nmega):
                i2 = gip.tile([128, E // 16], i16)
                nc.sync.dma_start(i2[:], gi[m])
                nut = nuvp.tile([H, cfg.mega], bf16, tag="nut")
                nc.sync.dma_start(nut[:], nu[m])
                vtt = nuvp.tile([H, cfg.mega], bf16, tag="vtt")
                nc.sync.dma_start(vtt[:], vt[m])

                slab = slabp.tile([128, E // 128, H], bf16)
                for j in range(E // NI):
                    nc.gpsimd.dma_gather(
                        slab[:, j * (NI // 128):(j + 1) * (NI // 128), :],
                        comp[m, :, :],
                        i2[:, j * (NI // 16):(j + 1) * (NI // 16)],
                        num_idxs=NI,
                        num_idxs_reg=NI,
                        elem_size=H,
                        queue_num=j % 4,
                    )
                slabT = slabTp.tile([128, E], bf16)
                for h in range(2):
                    nc.sync.dma_start_transpose(
                        slabT[:, h * (E // 2):(h + 1) * (E // 2)].rearrange(
                            "p (a b) -> p a b", b=H),
                        slab[:, h * (E // 256):(h + 1) * (E // 256), :]
                        .rearrange("p a b -> p (a b)"),
                    )
                if dump and m == 0:
                    nc.sync.dma_start(d_slabT[:, :], slabT[:])

                mx = mxp.tile([H, cfg.mega], f32)
                for g in range(E // MMC):
                    g1t = g1tp.tile([128, MMC], bf16)
                    nu2 = nut[:, (MMC // K) * g:(MMC // K) * (g + 1)]
                    nu3 = bass.AP(nu2.tensor, nu2.offset,
                                  list(nu2.ap) + [(0, K)])
                    nc.vector.tensor_max(
                        g1t[:].rearrange("p (a b) -> p a b", b=K),
                        slabT[:, MMC * g:MMC * (g + 1)].rearrange(
                            "p (a b) -> p a b", b=K),
                        nu3)
                    if dump and m == 0 and g == 0:
                        nc.sync.dma_start(d_g1t[:, :], g1t[:])
                    p2 = psp.tile([H, MMC], f32)
                    nc.tensor.matmul(
                        p2[:], lhsT=w2s[:], rhs=g1t[:], start=True, stop=True,
                    )
                    nc.vector.tensor_reduce(
                        out=mx[:, (MMC // K) * g:(MMC // K) * (g + 1)],
                        in_=p2[:].rearrange("p (a b) -> p a b", b=K),
                        axis=mybir.AxisListType.X,
                        op=mybir.AluOpType.max,
                    )

                outt = outp.tile([H, cfg.mega], bf16)
                nc.vector.tensor_tensor(
                    out=outt[:], in0=mx[:], in1=vtt[:],
                    op=mybir.AluOpType.add,
                )
                nc.sync.dma_start(
                    out2[:, m * cfg.mega:(m + 1) * cfg.mega], outt[:]
                )

    nc.compile()
    return nc


def host_prep(cfg: Cfg, x, ind, W1, b1, W2, b2):
    """Shared (core-independent) input prep. Returns (ptab, negu, v, w2b)."""
    x = np.asarray(x, np.float32)
    W1 = np.asarray(W1, np.float32)
    W2 = np.asarray(W2, np.float32)
    b1 = np.asarray(b1, np.float32)
    b2 = np.asarray(b2, np.float32)
    P = (x @ W1[:D]).astype(BF16)                        # [n, H] gather table
    u = x @ (W1[D:] - W1[:D]) + b1                       # [n, H] fp32
    negu = (-u).astype(BF16)                             # [n, H]
    v = (u @ W2 + b2).astype(np.float32)                 # [n, H]
    w2b = np.ascontiguousarray(W2.astype(BF16))
    return np.ascontiguousarray(P), negu, v, w2b


def _wrap_idx(idxs):
    """[..., n] int -> dma_gather idx tile [..., 128, n/16] int16
    (idx i at [i%16, i//16], replicated across the 8 partition groups)."""
    n = idxs.shape[-1]
    w = idxs.reshape(*idxs.shape[:-1], n // 16, 16)
    w = np.moveaxis(w, -1, -2)                  # [..., 16, n/16]
    return np.ascontiguousarray(
        np.broadcast_to(
            w[..., None, :, :],
            (*idxs.shape[:-1], 8, 16, n // 16),
        ).reshape(*idxs.shape[:-1], 128, n // 16).astype(np.int16))


def core_tables(cfg: Cfg, ptab, ind32, lo, hi):
    """Per-megablock compact tables + local idx for point range [lo, hi)."""
    E = cfg.e
    indc = np.zeros((cfg.npp, K), np.int32)
    indc[:hi - lo] = ind32[lo:hi]
    flat = indc.reshape(cfg.nmega, E)
    uniqs, ranks = [], np.empty((cfg.nmega, E), np.int64)
    for m in range(cfg.nmega):
        u_m, inv = np.unique(flat[m], return_inverse=True)
        uniqs.append(u_m)
        ranks[m] = inv
    return uniqs, ranks


def core_inputs(cfg: Cfg, ptab, negu, v, w2b, uniqs, ranks, lo, hi):
    comp = np.zeros((cfg.nmega, cfg.ct, H), BF16)
    for m in range(cfg.nmega):
        comp[m, :len(uniqs[m])] = ptab[uniqs[m]]
    gi = _wrap_idx(ranks)                                # [m, 128, E/16]

    nuc = np.zeros((cfg.npp, H), BF16)
    nuc[:hi - lo] = negu[lo:hi]
    nu = np.ascontiguousarray(
        nuc.reshape(cfg.nmega, cfg.mega, H).transpose(0, 2, 1))
    vc = np.zeros((cfg.npp, H), BF16)
    vc[:hi - lo] = v[lo:hi].astype(BF16)
    vtc = np.ascontiguousarray(
        vc.reshape(cfg.nmega, cfg.mega, H).transpose(0, 2, 1))
    return {"comp": comp, "gi": gi, "nu": nu, "vt": vtc, "w2": w2b}


_NC_CACHE = {}


def kernel(x, ind, W1, b1, W2, b2):
    from concourse import bass_utils

    ind32 = np.asarray(ind).astype(np.int32)
    cfg = Cfg()
    ptab, negu, v, w2b = host_prep(cfg, x, ind32, W1, b1, W2, b2)

    per_core = []
    maxct = 0
    for c in range(NCORES):
        lo = c * NP
        hi = min(lo + NP, N)
        uniqs, ranks = core_tables(cfg, ptab, ind32, lo, hi)
        per_core.append((uniqs, ranks, lo, hi))
        maxct = max(maxct, max(len(u) for u in uniqs))
    cfg.ct = maxct

    key = (cfg.n, cfg.np, cfg.mega, cfg.ct)
    if key not in _NC_CACHE:
        _NC_CACHE[key] = build_program(cfg)
    nc = _NC_CACHE[key]

    in_maps = []
    for uniqs, ranks, lo, hi in per_core:
        in_maps.append(
            core_inputs(cfg, ptab, negu, v, w2b, uniqs, ranks, lo, hi))

    res = bass_utils.run_bass_kernel_spmd(nc, in_maps, core_ids=list(range(NCORES)))
    out = np.empty((N, H), np.float32)
    for c in range(NCORES):
        lo = c * NP
        hi = min(lo + NP, N)
        out[lo:hi] = res.results[c]["out2"].T[:hi - lo].astype(np.float32)
    return out
